# revision 68
# baseline (speedup 1.0000x reference)
"""CSWin attention kernel for 8 trn2 NeuronCores — v12 (~150us, was 292us).

Data-parallel over B: 2 images = 16 windows per core, no collectives.

  - host pre-casts qkv to f16 AND pre-transposes per window:
      qT/kT   [c=128, win*512]      (channel-major, contiguous big DMAs)
      vtok    [tok128, win, jc, h, 33]  (token-major AV stationary with a
                                     ones column appended per head)
      vpad    [c, win, 662]         (host-padded LePE image, zero ring)
    -> no on-device casts, no PE transposes, 16 big contiguous input
       DMAs instead of 192 strided per-chunk DMAs.
  - QK^T per chunk-half into PSUM f32 [128,1024] (2 heads row-tiled).
  - exp split across engines (the PE runs matmuls strictly serially, so
    ScalarE exp @1elem/lane/cyc would otherwise be a 131us floor):
    ScalarE activation(Exp) for 6 of 8 half-tiles per window, DVE
    Schraudolph bit-trick (i16 = s*A+B viewed as f16) for {1,6} —
    chunks 0 and 3 exp on both engines in parallel.
  - AV+den fused: per head one [K=128, M=33] matmul ([v | ones]
    stationary, 64-wide PE tile) -> row 32 of each tile IS the softmax
    denominator; the 256 separate den ones-matmuls are gone.
  - LePE 3x3 depthwise conv: 6 diagonal-matmul taps on the PE (strided
    moving view, exactly 512 interior cols, two alternating PSUM-bank
    chains) + 3 taps on the DVE (scalar_tensor_tensor chain, SBUF f16);
    conv bias added on host.
  - inputs land via graded tiles (1,1,2,12 windows) so window 0 starts
    ~11us in instead of waiting ~26us for the full 8.9MB input DMA;
    the last window's lepe combine + output DMAs are not deferred into
    the drain tail.
  - outputs copied PSUM->SBUF f16 on DVE (GPSIMD cannot touch PSUM,
    DMA cannot read PSUM) and DMA'd f16; host does att = av/den +
    lepe + bias and the window->image transpose.

PSUM: scores 2x[128,1024](4 banks) + avA(1) + avB(1) + lpA(1) + lpB(1) = 8.
"""

import numpy as np

import concourse.bass as bass
import concourse.tile as tile
from concourse import mybir
from concourse.vector_clock import ScopedClock

RES = 64
SPLIT = 8
C = 128
HEADS = 4
HD = 32
S = RES * SPLIT          # 512 tokens per window
SCALE = HD ** -0.5
B = 16
N_CORES = 8
IMGS_PER_CORE = B // N_CORES   # 2
NWIN_IMG = RES // SPLIT        # 8 windows per image
NW = IMGS_PER_CORE * NWIN_IMG  # 16 windows per core
NCHUNK = S // 128              # 4 token-chunks per window

F32 = mybir.dt.float32
F16 = mybir.dt.float16
I16 = mybir.dt.int16

LOG2E = 1.4426950408889634
# Schraudolph f16: i16 = s * A + B, bit pattern of ~exp(s*SCALE)
SCH_A = float(SCALE * LOG2E * 1024.0)
SCH_B = float(15 * 1024 - 45 + 0.5)

# LePE padded image geometry (host-built): cell(y,x) = 12 + y*10 + x
XP = SPLIT + 2                 # 10
PADN = XP * (RES + 2) + 2      # 662
P0 = 12

# which half-tiles (idx = 2*jc + half, 0..7) use the DVE Schraudolph exp
DVE_HALVES = frozenset({1, 6})
TAPS = [(dy, dx) for dy in (-1, 0, 1) for dx in (-1, 0, 1)]
# LePE taps computed on the DVE (scalar_tensor_tensor chain) vs the PE
DVE_TAPS = (0, 1, 2)
PE_TAPS = tuple(t for t in range(9) if t not in DVE_TAPS)


# ---------------------------------------------------------------- compat ----

def _patched_drain_and_barrier(self, tick_clock, wait_clock):
    nc = self.nc
    nop_inst = nc.sync.nop(nofuse=True)
    wait_clock.add_sem_waits(nop_inst.ins, ScopedClock({None: tick_clock.global_clock}))
    si = nop_inst.ins.sync_info
    waits = list(si.on_wait) if si is not None else []
    if len(waits) > 1:
        si.on_wait = [waits[0]]
        for w in waits[1:]:
            n2 = nc.sync.nop(nofuse=True)
            n2.ins.sync_info = mybir.SyncInfo(on_wait=[w], on_update=[])
    nc.sync.drain()
    nc.all_engine_barrier()
    assert self.sems is not None
    popped = nc._tile_sem_poison_stack.pop()
    assert popped is self._sem_poison
    nc.clear_and_free_semaphores(list(self.sems.allocated().values()))
    nc.all_engine_barrier()


def _install_tile_patch():
    tile.TileContext._drain_and_barrier = _patched_drain_and_barrier


def _split_multiwaits(nc):
    """Hoist extra sync waits onto same-engine NOPs inserted just before the
    owning instruction (this walrus build allows 1 wait per instruction)."""
    for f in nc.m.functions:
        for bb in f.blocks:
            insts = bb.instructions
            if not any(
                i.sync_info is not None and len(i.sync_info.on_wait) > 1
                for i in insts
            ):
                continue
            new_insts = []
            for inst in insts:
                si = inst.sync_info
                if si is not None and len(si.on_wait) > 1:
                    waits = list(si.on_wait)
                    for w in waits[:-1]:
                        nop = mybir.InstNoOp(
                            name=nc.get_next_instruction_name(), ins=[], outs=[]
                        )
                        nop.engine = inst.engine
                        nop.sync_info = mybir.SyncInfo(on_wait=[w], on_update=[])
                        new_insts.append(nop)
                    si.on_wait = [waits[-1]]
                new_insts.append(inst)
            bb.instructions = new_insts


# ---------------------------------------------------------------- device ----

def _build_nc():
    _install_tile_patch()
    nc = bass.Bass(trn_type="TRN2", num_devices=N_CORES)

    AUGW = HD + 1            # 33: v columns + ones column (den rides along)
    AUGB = 2 * AUGW          # 66 aug columns per (chunk, bank)

    qT_d = nc.dram_tensor("qT", [C, NW * S], F16, kind="ExternalInput")
    kT_d = nc.dram_tensor("kT", [C, NW * S], F16, kind="ExternalInput")
    vtok_d = nc.dram_tensor(
        "vtok", [128, NW * NCHUNK * 2 * AUGB], F16, kind="ExternalInput"
    )
    vpad_d = nc.dram_tensor("vpad", [C, NW * PADN], F16, kind="ExternalInput")
    diag_d = nc.dram_tensor("diag", [C, 9 * C], F16, kind="ExternalInput")

    avA_d = nc.dram_tensor("avA", [NW, 98, S], F16, kind="ExternalOutput")
    avB_d = nc.dram_tensor("avB", [NW, 98, S], F16, kind="ExternalOutput")
    lp_d = nc.dram_tensor("lepeT", [NW, C, S], F16, kind="ExternalOutput")

    wtap_d = nc.dram_tensor("wtap", [C, 9], F32, kind="ExternalInput")

    GRP = 4  # windows per input-DMA group

    with tile.TileContext(nc) as tc:
        with (
            tc.tile_pool(name="const", bufs=1) as const,
            tc.tile_pool(name="inp", bufs=1) as inp,
            tc.tile_pool(name="expt", bufs=8) as expt,
            tc.tile_pool(name="sbout", bufs=3) as sbout,
            tc.tile_pool(name="lacc", bufs=3) as lacc,
            tc.tile_pool(name="scoresp", bufs=2, space="PSUM") as scoresp,
            tc.tile_pool(name="avp", bufs=1, space="PSUM") as avp,
            tc.tile_pool(name="lepep", bufs=1, space="PSUM") as lepep,
        ):
            # graded input tiles: [2, 2, 12] windows per tensor so window 0
            # can start on ~0.5MB of data; later groups land well before
            # their windows are reached. diag/wtap loads go AFTER the first
            # group so they don't delay it.
            GRADES = (1, 1, 2, NW - 4)
            specs = (
                ("qT", qT_d, S),
                ("kT", kT_d, S),
                ("vtok", vtok_d, NCHUNK * 2 * AUGB),
                ("vpad", vpad_d, PADN),
            )
            grp_t = {name: [] for name, _, _ in specs}
            starts = [sum(GRADES[:i]) for i in range(len(GRADES))]
            for gi, gw in enumerate(GRADES):
                for name, t_d, width in specs:
                    t = inp.tile(
                        [128, gw * width], F16, tag=f"{name}{gi}",
                        name=f"{name}{gi}",
                    )
                    lo = starts[gi] * width
                    nc.sync.dma_start(
                        out=t, in_=t_d.ap()[:, lo : lo + gw * width]
                    )
                    grp_t[name].append(t)
                if gi == 0:
                    diag_sb = const.tile([C, 9 * C], F16)
                    nc.sync.dma_start(out=diag_sb, in_=diag_d.ap())
                    wtap_sb = const.tile([C, 9], F32)
                    nc.sync.dma_start(out=wtap_sb, in_=wtap_d.ap())

            def in_sl(name, w, width, lo, hi, p0=0, p1=128):
                for gi, gw in enumerate(GRADES):
                    if w < starts[gi] + gw:
                        t = grp_t[name][gi]
                        off = (w - starts[gi]) * width
                        return t[p0:p1, off + lo : off + hi]
                raise AssertionError

            def vpad_view(w, t):
                dy, dx = TAPS[t]
                d = XP * dy + dx
                for gi, gw in enumerate(GRADES):
                    if w < starts[gi] + gw:
                        vp = grp_t["vpad"][gi]
                        wo = (w - starts[gi]) * PADN
                        break
                return bass.AP(
                    tensor=vp.tensor,
                    offset=vp.offset + wo + P0 + d,
                    ap=[vp.ap[0], [XP, RES], [1, SPLIT]],
                )

            def lepe_taps(w):
                """PE diag-matmul taps into TWO alternating PSUM banks (so
                consecutive taps have no accumulate RAW chain) + DVE STT taps
                into an SBUF f16 accumulator; returns (lpA, lpB, acc)."""
                lps = (
                    lepep.tile([128, S], F32, tag="lp", name="lpA"),
                    lepep.tile([128, S], F32, tag="lpB", name="lpB"),
                )
                n = len(PE_TAPS)
                for i, t in enumerate(PE_TAPS):
                    nc.tensor.matmul(
                        lps[i % 2],
                        diag_sb[:, 128 * t : 128 * (t + 1)],
                        vpad_view(w, t),
                        start=(i < 2),
                        stop=(i >= n - 2),
                        skip_group_check=True,
                    )
                acc = lacc.tile([128, S], F16, tag="lacc")
                for i, t in enumerate(DVE_TAPS):
                    if i == 0:
                        nc.vector.tensor_scalar(
                            out=acc,
                            in0=vpad_view(w, t),
                            scalar1=wtap_sb[:, t : t + 1],
                            scalar2=None,
                            op0=mybir.AluOpType.mult,
                        )
                    else:
                        nc.vector.scalar_tensor_tensor(
                            out=acc,
                            in0=vpad_view(w, t),
                            scalar=wtap_sb[:, t : t + 1],
                            in1=acc,
                            op0=mybir.AluOpType.mult,
                            op1=mybir.AluOpType.add,
                        )
                return lps[0], lps[1], acc

            def lepe_out(w, lpA, lpB, acc):
                # lepe = PE part (two PSUM f32 chains) + DVE part (SBUF f16)
                lp_sb = sbout.tile([128, S], F16, tag="lp_sb")
                nc.vector.scalar_tensor_tensor(
                    out=lp_sb,
                    in0=lpA,
                    scalar=1.0,
                    in1=acc,
                    op0=mybir.AluOpType.mult,
                    op1=mybir.AluOpType.add,
                )
                nc.vector.scalar_tensor_tensor(
                    out=lp_sb,
                    in0=lpB,
                    scalar=1.0,
                    in1=lp_sb,
                    op0=mybir.AluOpType.mult,
                    op1=mybir.AluOpType.add,
                )
                nc.gpsimd.dma_start(out=lp_d.ap()[w], in_=lp_sb)

            prev_lp = None  # (w, lpA, lpB, acc) of the previous window

            for w in range(NW):
                avA_ps = avp.tile([128, S], F32, tag="avA")
                avB_ps = avp.tile([128, S], F32, tag="avB")
                ets = {}

                def qk_exp(jc, half, w=w, ets=None):
                    st = scoresp.tile([128, 2 * S], F32, tag="st")
                    for hh in range(2):
                        h = 2 * half + hh
                        hp = 32 * h
                        nc.tensor.matmul(
                            st[:, S * hh : S * (hh + 1)],
                            in_sl("kT", w, S, 128 * jc, 128 * (jc + 1), hp, hp + 32),
                            in_sl("qT", w, S, 0, S, hp, hp + 32),
                            start=True,
                            stop=True,
                            tile_position=(hp, 0),
                        )
                    et = expt.tile([128, 2 * S], F16, tag="et")
                    idx = 2 * jc + half
                    if idx in DVE_HALVES:
                        nc.vector.tensor_scalar(
                            out=et.bitcast(I16),
                            in0=st,
                            scalar1=SCH_A,
                            scalar2=SCH_B,
                            op0=mybir.AluOpType.mult,
                            op1=mybir.AluOpType.add,
                        )
                    else:
                        nc.scalar.activation(
                            out=et,
                            in_=st,
                            func=mybir.ActivationFunctionType.Exp,
                            scale=float(SCALE),
                        )
                    ets[idx] = et

                def av_den(jc, w=w, ets=None):
                    # per head one [K=128, M=33] matmul: v columns + a ones
                    # column, so row 32 of each 64-wide PE tile is the
                    # softmax denominator (no separate den matmuls)
                    base = jc * 2 * AUGB
                    for h in range(HEADS):
                        bank = avA_ps if h < 2 else avB_ps
                        tc_ = 64 * (h % 2)
                        et_h = ets[2 * jc + h // 2][:, S * (h % 2) : S * (h % 2 + 1)]
                        nc.tensor.matmul(
                            bank[tc_ : tc_ + AUGW, :],
                            in_sl(
                                "vtok", w, NCHUNK * 2 * AUGB,
                                base + AUGW * h, base + AUGW * (h + 1),
                            ),
                            et_h,
                            start=(jc == 0),
                            stop=(jc == NCHUNK - 1),
                            tile_position=(0, tc_),
                            skip_group_check=True,
                        )

                qk_exp(0, 0, ets=ets)
                qk_exp(0, 1, ets=ets)
                qk_exp(1, 0, ets=ets)
                qk_exp(1, 1, ets=ets)
                av_den(0, ets=ets)
                qk_exp(2, 0, ets=ets)
                qk_exp(2, 1, ets=ets)
                av_den(1, ets=ets)
                qk_exp(3, 0, ets=ets)
                qk_exp(3, 1, ets=ets)
                av_den(2, ets=ets)

                # drain previous window's lepe bank, then fill it for w;
                # the taps cover the PE while exp(3,*) drains
                if prev_lp is not None:
                    lepe_out(*prev_lp)
                prev_lp = (w, *lepe_taps(w))

                av_den(3, ets=ets)

                last = w == NW - 1
                if last:
                    # don't defer the final lepe combine into the drain tail
                    lepe_out(*prev_lp)
                    prev_lp = None
                out_q = nc.sync if last else nc.gpsimd

                # ---- drain PSUM -> SBUF f16 -> DRAM ------------------------
                sbA = sbout.tile([128, S], F16, tag="sbA")
                nc.vector.tensor_copy(out=sbA[0:98, :], in_=avA_ps[0:98, :])
                out_q.dma_start(out=avA_d.ap()[w], in_=sbA[0:98, :])

                sbB = sbout.tile([128, S], F16, tag="sbB")
                nc.vector.tensor_copy(out=sbB[0:98, :], in_=avB_ps[0:98, :])
                out_q.dma_start(out=avB_d.ap()[w], in_=sbB[0:98, :])

    _split_multiwaits(nc)
    return nc


# ------------------------------------------------------------------ host ----

_NC_CACHE = {}


def _get_nc():
    if "nc" not in _NC_CACHE:
        _NC_CACHE["nc"] = _build_nc()
    return _NC_CACHE["nc"]


def _host_prep(qkv, conv_w):
    """Build per-core input arrays (all f16)."""
    f16 = np.float16
    # [3, B, 4096, 128] -> window grids [3, B, y, sx, x, c]
    qkv_w = qkv.reshape(3, B, RES, NWIN_IMG, SPLIT, C)

    cores = []
    for core in range(N_CORES):
        bs = slice(core * IMGS_PER_CORE, (core + 1) * IMGS_PER_CORE)
        q = qkv_w[0, bs]   # [2, y, sx, x, c]
        k = qkv_w[1, bs]
        v = qkv_w[2, bs]

        # [c, img, sx, y, x] -> [128, NW*512]
        qT = np.ascontiguousarray(q.transpose(4, 0, 2, 1, 3)).reshape(C, NW * S)
        kT = np.ascontiguousarray(k.transpose(4, 0, 2, 1, 3)).reshape(C, NW * S)

        # v token-major, augmented with a ones column per head:
        # layout [tok128, win, jc, h, 33] with cols = [v_h (32) | 1]
        vt = v.reshape(IMGS_PER_CORE, NCHUNK, 16, NWIN_IMG, SPLIT, C)
        vtok = np.ascontiguousarray(vt.transpose(2, 4, 0, 3, 1, 5)).reshape(
            128, NW, NCHUNK, HEADS, HD
        )
        vaug = np.ones((128, NW, NCHUNK, HEADS, HD + 1), dtype=np.float32)
        vaug[..., :HD] = vtok
        vaug = vaug.reshape(128, NW * NCHUNK * HEADS * (HD + 1))

        # vpad: [c, win, 662] with interior at 12 + y*10 + x
        vimg = np.ascontiguousarray(v.transpose(4, 0, 2, 1, 3))  # [c, img, sx, y, x]
        vpad = np.zeros((C, NW, PADN), dtype=f16)
        vpad_v = vpad[:, :, 1:661].reshape(C, NW, RES + 2, XP)
        vpad_v[:, :, 1:-1, 1:-1] = vimg.reshape(C, NW, RES, SPLIT)

        cores.append(
            {
                "qT": qT.astype(f16),
                "kT": kT.astype(f16),
                "vtok": vaug.astype(f16),
                "vpad": vpad.reshape(C, NW * PADN),
            }
        )

    w9 = conv_w.reshape(C, 9).astype(np.float32)
    diag = np.zeros((C, 9, C), dtype=np.float32)
    idx = np.arange(C)
    for t in range(9):
        diag[idx, t, idx] = w9[:, t]
    diag = diag.reshape(C, 9 * C).astype(f16)
    for m in cores:
        m["diag"] = diag
        m["wtap"] = w9
    return cores


def kernel(qkv, conv_w, conv_b):
    from concourse.bass_utils import run_bass_kernel_spmd

    qkv = np.asarray(qkv, dtype=np.float32)
    conv_w = np.asarray(conv_w, np.float32)
    conv_b = np.asarray(conv_b, np.float32)

    nc = _get_nc()
    in_maps = _host_prep(qkv, conv_w)

    res = run_bass_kernel_spmd(nc, in_maps, core_ids=list(range(N_CORES)))
    global LAST_RESULT
    LAST_RESULT = res

    outs = []
    for r in res.results:
        avA = r["avA"].astype(np.float32)      # [16, 98, 512]
        avB = r["avB"].astype(np.float32)
        lp = r["lepeT"].astype(np.float32)     # [16, 128, 512]
        att = np.empty((NW, HEADS, HD, S), np.float32)
        for h, (bank, row) in enumerate(
            ((avA, 0), (avA, 64), (avB, 0), (avB, 64))
        ):
            att[:, h] = bank[:, row : row + HD] / bank[:, None, row + HD]
        o = att.reshape(NW, C, S) + lp + conv_b.astype(np.float32)[None, :, None]
        # [win, c, s] -> [img, y, x, c]
        o = o.reshape(IMGS_PER_CORE, NWIN_IMG, C, RES, SPLIT)
        o = o.transpose(0, 3, 1, 4, 2).reshape(IMGS_PER_CORE, RES, RES, C)
        outs.append(o)
    return np.concatenate(outs, axis=0)


LAST_RESULT = None


# revision 69
# speedup vs baseline: 1.2771x; 1.2771x over previous
"""CSWin attention kernel for 8 trn2 NeuronCores — v12 (~150us, was 292us).

Data-parallel over B: 2 images = 16 windows per core, no collectives.

  - host pre-casts qkv to f16 AND pre-transposes per window:
      qT/kT   [c=128, win*512]      (channel-major, contiguous big DMAs)
      vtok    [tok128, win, jc, h, 33]  (token-major AV stationary with a
                                     ones column appended per head)
      vpad    [c, win, 662]         (host-padded LePE image, zero ring)
    -> no on-device casts, no PE transposes, 16 big contiguous input
       DMAs instead of 192 strided per-chunk DMAs.
  - QK^T per chunk-half into PSUM f32 [128,1024] (2 heads row-tiled).
  - exp split across engines (the PE runs matmuls strictly serially, so
    ScalarE exp @1elem/lane/cyc would otherwise be a 131us floor):
    ScalarE activation(Exp) for 6 of 8 half-tiles per window, DVE
    Schraudolph bit-trick (i16 = s*A+B viewed as f16) for {1,6} —
    chunks 0 and 3 exp on both engines in parallel.
  - AV+den fused: per head one [K=128, M=33] matmul ([v | ones]
    stationary, 64-wide PE tile) -> row 32 of each tile IS the softmax
    denominator; the 256 separate den ones-matmuls are gone.
  - LePE 3x3 depthwise conv: 6 diagonal-matmul taps on the PE (strided
    moving view, exactly 512 interior cols, two alternating PSUM-bank
    chains) + 3 taps on the DVE (scalar_tensor_tensor chain, SBUF f16);
    conv bias added on host.
  - inputs land via graded tiles (1,1,2,12 windows) so window 0 starts
    ~11us in instead of waiting ~26us for the full 8.9MB input DMA;
    the last window's lepe combine + output DMAs are not deferred into
    the drain tail.
  - outputs copied PSUM->SBUF f16 on DVE (GPSIMD cannot touch PSUM,
    DMA cannot read PSUM) and DMA'd f16; host does att = av/den +
    lepe + bias and the window->image transpose.

PSUM: scores 2x[128,1024](4 banks) + avA(1) + avB(1) + lpA(1) + lpB(1) = 8.
"""

import numpy as np

import concourse.bass as bass
import concourse.tile as tile
from concourse import mybir
from concourse.vector_clock import ScopedClock

RES = 64
SPLIT = 8
C = 128
HEADS = 4
HD = 32
S = RES * SPLIT          # 512 tokens per window
SCALE = HD ** -0.5
B = 16
N_CORES = 8
IMGS_PER_CORE = B // N_CORES   # 2
NWIN_IMG = RES // SPLIT        # 8 windows per image
NW = IMGS_PER_CORE * NWIN_IMG  # 16 windows per core
NCHUNK = S // 128              # 4 token-chunks per window

F32 = mybir.dt.float32
F16 = mybir.dt.float16
I16 = mybir.dt.int16

LOG2E = 1.4426950408889634
# Schraudolph f16: i16 = s * A + B, bit pattern of ~exp(s*SCALE)
SCH_A = float(SCALE * LOG2E * 1024.0)
SCH_B = float(15 * 1024 - 45 + 0.5)

# LePE padded image geometry (host-built): cell(y,x) = 12 + y*10 + x
XP = SPLIT + 2                 # 10
PADN = XP * (RES + 2) + 2      # 662
P0 = 12

# which half-tiles (idx = 2*jc + half, 0..7) use the DVE Schraudolph exp
DVE_HALVES = frozenset({1, 6})
TAPS = [(dy, dx) for dy in (-1, 0, 1) for dx in (-1, 0, 1)]
# LePE taps computed on the DVE (scalar_tensor_tensor chain) vs the PE
DVE_TAPS = (0, 1, 2)
PE_TAPS = tuple(t for t in range(9) if t not in DVE_TAPS)


# ---------------------------------------------------------------- compat ----

def _patched_drain_and_barrier(self, tick_clock, wait_clock):
    nc = self.nc
    nop_inst = nc.sync.nop(nofuse=True)
    wait_clock.add_sem_waits(nop_inst.ins, ScopedClock({None: tick_clock.global_clock}))
    si = nop_inst.ins.sync_info
    waits = list(si.on_wait) if si is not None else []
    if len(waits) > 1:
        si.on_wait = [waits[0]]
        for w in waits[1:]:
            n2 = nc.sync.nop(nofuse=True)
            n2.ins.sync_info = mybir.SyncInfo(on_wait=[w], on_update=[])
    nc.sync.drain()
    nc.all_engine_barrier()
    assert self.sems is not None
    popped = nc._tile_sem_poison_stack.pop()
    assert popped is self._sem_poison
    nc.clear_and_free_semaphores(list(self.sems.allocated().values()))
    nc.all_engine_barrier()


def _install_tile_patch():
    tile.TileContext._drain_and_barrier = _patched_drain_and_barrier


def _split_multiwaits(nc):
    """Hoist extra sync waits onto same-engine NOPs inserted just before the
    owning instruction (this walrus build allows 1 wait per instruction)."""
    for f in nc.m.functions:
        for bb in f.blocks:
            insts = bb.instructions
            if not any(
                i.sync_info is not None and len(i.sync_info.on_wait) > 1
                for i in insts
            ):
                continue
            new_insts = []
            for inst in insts:
                si = inst.sync_info
                if si is not None and len(si.on_wait) > 1:
                    waits = list(si.on_wait)
                    for w in waits[:-1]:
                        nop = mybir.InstNoOp(
                            name=nc.get_next_instruction_name(), ins=[], outs=[]
                        )
                        nop.engine = inst.engine
                        nop.sync_info = mybir.SyncInfo(on_wait=[w], on_update=[])
                        new_insts.append(nop)
                    si.on_wait = [waits[-1]]
                new_insts.append(inst)
            bb.instructions = new_insts


# ---------------------------------------------------------------- device ----

def _build_nc():
    _install_tile_patch()
    nc = bass.Bass(trn_type="TRN2", num_devices=N_CORES)

    AUGW = HD + 1            # 33: v columns + ones column (den rides along)
    AUGB = 2 * AUGW          # 66 aug columns per (chunk, bank)

    qT_d = nc.dram_tensor("qT", [C, NW * S], F16, kind="ExternalInput")
    kT_d = nc.dram_tensor("kT", [C, NW * S], F16, kind="ExternalInput")
    vtok_d = nc.dram_tensor(
        "vtok", [128, NW * NCHUNK * 2 * AUGB], F16, kind="ExternalInput"
    )
    vpad_d = nc.dram_tensor("vpad", [C, NW * PADN], F16, kind="ExternalInput")
    diag_d = nc.dram_tensor("diag", [C, 9 * C], F16, kind="ExternalInput")

    avA_d = nc.dram_tensor("avA", [NW, 98, S], F16, kind="ExternalOutput")
    avB_d = nc.dram_tensor("avB", [NW, 98, S], F16, kind="ExternalOutput")
    lp_d = nc.dram_tensor("lepeT", [NW, C, S], F16, kind="ExternalOutput")

    wtap_d = nc.dram_tensor("wtap", [C, 9], F32, kind="ExternalInput")

    GRP = 4  # windows per input-DMA group

    with tile.TileContext(nc) as tc:
        with (
            tc.tile_pool(name="const", bufs=1) as const,
            tc.tile_pool(name="inp", bufs=1) as inp,
            tc.tile_pool(name="expt", bufs=6) as expt,
            tc.tile_pool(name="sbout", bufs=3) as sbout,
            tc.tile_pool(name="lacc", bufs=2) as lacc,
            tc.tile_pool(name="scoresp", bufs=2, space="PSUM") as scoresp,
            tc.tile_pool(name="avp", bufs=1, space="PSUM") as avp,
            tc.tile_pool(name="lepep", bufs=1, space="PSUM") as lepep,
        ):
            # graded input tiles: [2, 2, 12] windows per tensor so window 0
            # can start on ~0.5MB of data; later groups land well before
            # their windows are reached. diag/wtap loads go AFTER the first
            # group so they don't delay it.
            GRADES = (1, 1, 2, NW - 4)
            specs = (
                ("qT", qT_d, S),
                ("kT", kT_d, S),
                ("vtok", vtok_d, NCHUNK * 2 * AUGB),
                ("vpad", vpad_d, PADN),
            )
            grp_t = {name: [] for name, _, _ in specs}
            starts = [sum(GRADES[:i]) for i in range(len(GRADES))]
            for gi, gw in enumerate(GRADES):
                for name, t_d, width in specs:
                    t = inp.tile(
                        [128, gw * width], F16, tag=f"{name}{gi}",
                        name=f"{name}{gi}",
                    )
                    lo = starts[gi] * width
                    nc.sync.dma_start(
                        out=t, in_=t_d.ap()[:, lo : lo + gw * width]
                    )
                    grp_t[name].append(t)
                if gi == 0:
                    diag_sb = const.tile([C, 9 * C], F16)
                    nc.sync.dma_start(out=diag_sb, in_=diag_d.ap())
                    wtap_sb = const.tile([C, 9], F32)
                    nc.sync.dma_start(out=wtap_sb, in_=wtap_d.ap())

            def in_sl(name, w, width, lo, hi, p0=0, p1=128):
                for gi, gw in enumerate(GRADES):
                    if w < starts[gi] + gw:
                        t = grp_t[name][gi]
                        off = (w - starts[gi]) * width
                        return t[p0:p1, off + lo : off + hi]
                raise AssertionError

            def vpad_view(w, t):
                dy, dx = TAPS[t]
                d = XP * dy + dx
                for gi, gw in enumerate(GRADES):
                    if w < starts[gi] + gw:
                        vp = grp_t["vpad"][gi]
                        wo = (w - starts[gi]) * PADN
                        break
                return bass.AP(
                    tensor=vp.tensor,
                    offset=vp.offset + wo + P0 + d,
                    ap=[vp.ap[0], [XP, RES], [1, SPLIT]],
                )

            def lepe_taps(w):
                """PE diag-matmul taps into TWO alternating PSUM banks (so
                consecutive taps have no accumulate RAW chain) + DVE STT taps
                into an SBUF f16 accumulator; returns (lpA, lpB, acc)."""
                lps = (
                    lepep.tile([128, S], F32, tag="lp", name="lpA"),
                    lepep.tile([128, S], F32, tag="lpB", name="lpB"),
                )
                n = len(PE_TAPS)
                for i, t in enumerate(PE_TAPS):
                    nc.tensor.matmul(
                        lps[i % 2],
                        diag_sb[:, 128 * t : 128 * (t + 1)],
                        vpad_view(w, t),
                        start=(i < 2),
                        stop=(i >= n - 2),
                        skip_group_check=True,
                    )
                acc = lacc.tile([128, S], F16, tag="lacc")
                for i, t in enumerate(DVE_TAPS):
                    if i == 0:
                        nc.vector.tensor_scalar(
                            out=acc,
                            in0=vpad_view(w, t),
                            scalar1=wtap_sb[:, t : t + 1],
                            scalar2=None,
                            op0=mybir.AluOpType.mult,
                        )
                    else:
                        nc.vector.scalar_tensor_tensor(
                            out=acc,
                            in0=vpad_view(w, t),
                            scalar=wtap_sb[:, t : t + 1],
                            in1=acc,
                            op0=mybir.AluOpType.mult,
                            op1=mybir.AluOpType.add,
                        )
                return lps[0], lps[1], acc

            def lepe_out(w, lpA, lpB, acc):
                # lepe = PE part (two PSUM f32 chains) + DVE part (SBUF f16)
                lp_sb = sbout.tile([128, S], F16, tag="lp_sb")
                nc.vector.scalar_tensor_tensor(
                    out=lp_sb,
                    in0=lpA,
                    scalar=1.0,
                    in1=acc,
                    op0=mybir.AluOpType.mult,
                    op1=mybir.AluOpType.add,
                )
                nc.vector.scalar_tensor_tensor(
                    out=lp_sb,
                    in0=lpB,
                    scalar=1.0,
                    in1=lp_sb,
                    op0=mybir.AluOpType.mult,
                    op1=mybir.AluOpType.add,
                )
                nc.gpsimd.dma_start(out=lp_d.ap()[w], in_=lp_sb)

            prev_lp = None  # (w, lpA, lpB, acc) of the previous window

            for w in range(NW):
                avA_ps = avp.tile([128, S], F32, tag="avA")
                avB_ps = avp.tile([128, S], F32, tag="avB")
                ets = {}

                def qk_exp(jc, half, w=w, ets=None):
                    st = scoresp.tile([128, 2 * S], F32, tag="st")
                    for hh in range(2):
                        h = 2 * half + hh
                        hp = 32 * h
                        nc.tensor.matmul(
                            st[:, S * hh : S * (hh + 1)],
                            in_sl("kT", w, S, 128 * jc, 128 * (jc + 1), hp, hp + 32),
                            in_sl("qT", w, S, 0, S, hp, hp + 32),
                            start=True,
                            stop=True,
                            tile_position=(hp, 0),
                        )
                    et = expt.tile([128, 2 * S], F16, tag="et")
                    idx = 2 * jc + half
                    if idx in DVE_HALVES:
                        nc.vector.tensor_scalar(
                            out=et.bitcast(I16),
                            in0=st,
                            scalar1=SCH_A,
                            scalar2=SCH_B,
                            op0=mybir.AluOpType.mult,
                            op1=mybir.AluOpType.add,
                        )
                    else:
                        nc.scalar.activation(
                            out=et,
                            in_=st,
                            func=mybir.ActivationFunctionType.Exp,
                            scale=float(SCALE),
                        )
                    ets[idx] = et

                def av_den(jc, w=w, ets=None):
                    # per head one [K=128, M=33] matmul: v columns + a ones
                    # column, so row 32 of each 64-wide PE tile is the
                    # softmax denominator (no separate den matmuls)
                    base = jc * 2 * AUGB
                    for h in range(HEADS):
                        bank = avA_ps if h < 2 else avB_ps
                        tc_ = 64 * (h % 2)
                        et_h = ets[2 * jc + h // 2][:, S * (h % 2) : S * (h % 2 + 1)]
                        nc.tensor.matmul(
                            bank[tc_ : tc_ + AUGW, :],
                            in_sl(
                                "vtok", w, NCHUNK * 2 * AUGB,
                                base + AUGW * h, base + AUGW * (h + 1),
                            ),
                            et_h,
                            start=(jc == 0),
                            stop=(jc == NCHUNK - 1),
                            tile_position=(0, tc_),
                            skip_group_check=True,
                        )

                qk_exp(0, 0, ets=ets)
                qk_exp(0, 1, ets=ets)
                qk_exp(1, 0, ets=ets)
                qk_exp(1, 1, ets=ets)
                av_den(0, ets=ets)
                qk_exp(2, 0, ets=ets)
                qk_exp(2, 1, ets=ets)
                av_den(1, ets=ets)
                qk_exp(3, 0, ets=ets)
                qk_exp(3, 1, ets=ets)
                av_den(2, ets=ets)

                # drain previous window's lepe bank, then fill it for w;
                # the taps cover the PE while exp(3,*) drains
                if prev_lp is not None:
                    lepe_out(*prev_lp)
                prev_lp = (w, *lepe_taps(w))

                av_den(3, ets=ets)

                last = w == NW - 1
                if last:
                    # don't defer the final lepe combine into the drain tail
                    lepe_out(*prev_lp)
                    prev_lp = None
                out_q = nc.sync if last else nc.gpsimd

                # ---- drain PSUM -> SBUF f16 -> DRAM ------------------------
                sbA = sbout.tile([128, S], F16, tag="sbA")
                nc.vector.tensor_copy(out=sbA[0:98, :], in_=avA_ps[0:98, :])
                out_q.dma_start(out=avA_d.ap()[w], in_=sbA[0:98, :])

                sbB = sbout.tile([128, S], F16, tag="sbB")
                nc.vector.tensor_copy(out=sbB[0:98, :], in_=avB_ps[0:98, :])
                out_q.dma_start(out=avB_d.ap()[w], in_=sbB[0:98, :])

    _split_multiwaits(nc)
    return nc


# ------------------------------------------------------------------ host ----

_NC_CACHE = {}


def _get_nc():
    if "nc" not in _NC_CACHE:
        _NC_CACHE["nc"] = _build_nc()
    return _NC_CACHE["nc"]


def _host_prep(qkv, conv_w):
    """Build per-core input arrays (all f16)."""
    f16 = np.float16
    # [3, B, 4096, 128] -> window grids [3, B, y, sx, x, c]
    qkv_w = qkv.reshape(3, B, RES, NWIN_IMG, SPLIT, C)

    cores = []
    for core in range(N_CORES):
        bs = slice(core * IMGS_PER_CORE, (core + 1) * IMGS_PER_CORE)
        q = qkv_w[0, bs]   # [2, y, sx, x, c]
        k = qkv_w[1, bs]
        v = qkv_w[2, bs]

        # [c, img, sx, y, x] -> [128, NW*512]
        qT = np.ascontiguousarray(q.transpose(4, 0, 2, 1, 3)).reshape(C, NW * S)
        kT = np.ascontiguousarray(k.transpose(4, 0, 2, 1, 3)).reshape(C, NW * S)

        # v token-major, augmented with a ones column per head:
        # layout [tok128, win, jc, h, 33] with cols = [v_h (32) | 1]
        vt = v.reshape(IMGS_PER_CORE, NCHUNK, 16, NWIN_IMG, SPLIT, C)
        vtok = np.ascontiguousarray(vt.transpose(2, 4, 0, 3, 1, 5)).reshape(
            128, NW, NCHUNK, HEADS, HD
        )
        vaug = np.ones((128, NW, NCHUNK, HEADS, HD + 1), dtype=np.float32)
        vaug[..., :HD] = vtok
        vaug = vaug.reshape(128, NW * NCHUNK * HEADS * (HD + 1))

        # vpad: [c, win, 662] with interior at 12 + y*10 + x
        vimg = np.ascontiguousarray(v.transpose(4, 0, 2, 1, 3))  # [c, img, sx, y, x]
        vpad = np.zeros((C, NW, PADN), dtype=f16)
        vpad_v = vpad[:, :, 1:661].reshape(C, NW, RES + 2, XP)
        vpad_v[:, :, 1:-1, 1:-1] = vimg.reshape(C, NW, RES, SPLIT)

        cores.append(
            {
                "qT": qT.astype(f16),
                "kT": kT.astype(f16),
                "vtok": vaug.astype(f16),
                "vpad": vpad.reshape(C, NW * PADN),
            }
        )

    w9 = conv_w.reshape(C, 9).astype(np.float32)
    diag = np.zeros((C, 9, C), dtype=np.float32)
    idx = np.arange(C)
    for t in range(9):
        diag[idx, t, idx] = w9[:, t]
    diag = diag.reshape(C, 9 * C).astype(f16)
    for m in cores:
        m["diag"] = diag
        m["wtap"] = w9
    return cores


def kernel(qkv, conv_w, conv_b):
    from concourse.bass_utils import run_bass_kernel_spmd

    qkv = np.asarray(qkv, dtype=np.float32)
    conv_w = np.asarray(conv_w, np.float32)
    conv_b = np.asarray(conv_b, np.float32)

    nc = _get_nc()
    in_maps = _host_prep(qkv, conv_w)

    res = run_bass_kernel_spmd(nc, in_maps, core_ids=list(range(N_CORES)))
    global LAST_RESULT
    LAST_RESULT = res

    outs = []
    for r in res.results:
        avA = r["avA"].astype(np.float32)      # [16, 98, 512]
        avB = r["avB"].astype(np.float32)
        lp = r["lepeT"].astype(np.float32)     # [16, 128, 512]
        att = np.empty((NW, HEADS, HD, S), np.float32)
        for h, (bank, row) in enumerate(
            ((avA, 0), (avA, 64), (avB, 0), (avB, 64))
        ):
            att[:, h] = bank[:, row : row + HD] / bank[:, None, row + HD]
        o = att.reshape(NW, C, S) + lp + conv_b.astype(np.float32)[None, :, None]
        # [win, c, s] -> [img, y, x, c]
        o = o.reshape(IMGS_PER_CORE, NWIN_IMG, C, RES, SPLIT)
        o = o.transpose(0, 3, 1, 4, 2).reshape(IMGS_PER_CORE, RES, RES, C)
        outs.append(o)
    return np.concatenate(outs, axis=0)


LAST_RESULT = None
